# revision 1
# baseline (speedup 1.0000x reference)
"""1D horizontal correlation (FlowNet cost volume, kernel_size=1) on 8 TRN2 cores.

out[b, d+4, y, x] = mean_c x1[b,c,y,x] * x2[b,c,y,x+d],  d in [-4, 4], OOB -> 0

Strategy:
- Data-parallel over batch: B=8 -> one batch element per NeuronCore.
- Per core, flatten (H, W) -> S=30720 positions. C=128 = partition dim.
- For each 128-position tile t: TensorE band matmul (bf16 in, fp32 psum)
      psum[m, n] = sum_c x1[c, 128t+m] * x2[c, 128t-4+n],  n in [0, 136)
  The 9 needed outputs per position are the diagonals psum[m, m+j], j=0..8.
- ScalarE copies each psum tile to SBUF (bf16); tiles are batched and DMAed
  to DRAM; the host extracts the diagonal band (a numpy gather) and applies
  the zero mask for displacements that crossed a row boundary in the
  flattened layout.
- Inputs are host-cast to bf16 with the 1/C scale folded into x1 (exact:
  power of two), halving input DMA traffic.

The TRN2 walrus build here allows only ONE sync-wait per instruction, so the
kernel is shaped to never need two:
- x1/x2 live in persistent full-size SBUF tiles filled by disjoint slice
  DMAs (no buffer reuse -> DMAs carry no waits).
- outt buffers are one-per-chunk (no reuse -> ACT copies wait only on PE).
- Before each matmul whose PSUM slot is being recycled, a 1-column junk
  LDWEIGHTS reads the outt region written by the ACT copy that released
  that slot: the LDW absorbs the ACT wait into PE's observed clock, so the
  real matmul's PSUM-release wait is elided and it carries at most the
  one DMA wait for fresh x2 data.
"""

import os
import numpy as np

import concourse.bass as bass
import concourse.bacc as bacc
import concourse.mybir as mybir
import concourse.tile as tile
from concourse import bass_utils

B, C, H, W = 8, 128, 96, 320
S = H * W            # 30720 flattened positions per batch element
MAXD = 4
ND = 2 * MAXD + 1    # 9 displacement channels
TP = 128             # positions per tile (PSUM partition dim)
NT = S // TP         # 240 tiles
NB = TP + 2 * MAXD   # 136 band columns per tile
K = 30               # tiles per output chunk
NCHUNK = NT // K     # 8
NSLICE = 16          # input DMA slices per tensor
SLICE = S // NSLICE  # 1920 positions
PSB = 8              # psum pool bufs (= all 8 banks)
GRP = 3              # matmul outputs packed per psum bank (3*136*4B < 2KB)

F32 = mybir.dt.float32
BF16 = mybir.dt.bfloat16


def _build_nc():
    nc = bacc.Bacc(debug=False)
    x1 = nc.dram_tensor("x1", [C, S], BF16, kind="ExternalInput")
    # x2 is host-padded with a zero halo of MAXD on both ends: [C, S + 8];
    # dram/sbuf col j = position j - MAXD.
    x2 = nc.dram_tensor("x2", [C, S + 2 * MAXD], BF16, kind="ExternalInput")
    gram = nc.dram_tensor("gram", [NT, TP, NB], BF16, kind="ExternalOutput")

    with tile.TileContext(nc) as tc:
        with (
            tc.tile_pool(name="x1p", bufs=1) as x1p,
            tc.tile_pool(name="x2p", bufs=1) as x2p,
            tc.tile_pool(name="psp", bufs=PSB, space="PSUM") as psp,
            tc.tile_pool(name="outp", bufs=NCHUNK) as outp,
        ):
            x1full = x1p.tile([C, S], BF16)
            x2full = x2p.tile([C, S + 2 * MAXD], BF16)
            for i in range(NSLICE):
                lo, hi = i * SLICE, (i + 1) * SLICE
                nc.scalar.dma_start(out=x1full[:, lo:hi], in_=x1[:, lo:hi])
                xhi = hi + 2 * MAXD if i == NSLICE - 1 else hi
                nc.scalar.dma_start(out=x2full[:, lo:xhi], in_=x2[:, lo:xhi])

            for ci in range(NCHUNK):
                outt = outp.tile([TP, K * NB], BF16)
                for k in range(K):
                    t = ci * K + k
                    ps = psp.tile([TP, NB], F32)
                    nc.tensor.matmul(
                        ps[:],
                        lhsT=x1full[:, TP * t : TP * (t + 1)],
                        rhs=x2full[:, TP * t : TP * t + NB],
                        start=True,
                        stop=True,
                    )
                    nc.scalar.copy(outt[:, k * NB : (k + 1) * NB], ps[:])

                nc.sync.dma_start(
                    out=gram[ci * K : (ci + 1) * K].rearrange("k p n -> p k n"),
                    in_=outt[:].rearrange("p (k n) -> p k n", k=K),
                )
    nc.compile()
    return nc


_NC_CACHE = {}


def _get_nc():
    if "nc" not in _NC_CACHE:
        _NC_CACHE["nc"] = _build_nc()
    return _NC_CACHE["nc"]


# host-side diagonal gather indices: band[t, m, j] = gram[t, m, m + j]
_M_IDX = np.arange(TP)[:, None]
_J_IDX = np.arange(ND)[None, :]


def _extract(gram: np.ndarray) -> np.ndarray:
    """gram [NT, TP, NB] -> out [ND, H, W] with OOB zero mask applied."""
    band = np.asarray(gram[:, _M_IDX, _M_IDX + _J_IDX], dtype=np.float32)
    out = band.transpose(2, 0, 1).reshape(ND, H, W)  # out[j, y, x]
    out = np.ascontiguousarray(out)
    for j in range(ND):
        d = j - MAXD
        if d < 0:
            out[j, :, :-d] = 0.0
        elif d > 0:
            out[j, :, W - d :] = 0.0
    return out


def kernel(x1: np.ndarray, x2: np.ndarray) -> np.ndarray:
    assert x1.shape == (B, C, H, W) and x2.shape == (B, C, H, W)
    import ml_dtypes

    bf16 = ml_dtypes.bfloat16
    nc = _get_nc()
    # fold the 1/C mean scale into x1 (C = 128: exact exponent shift in bf16)
    x1b = (x1.reshape(B, C, S) * np.float32(1.0 / C)).astype(bf16)
    x2p = np.zeros((B, C, S + 2 * MAXD), dtype=bf16)
    x2p[:, :, MAXD : MAXD + S] = x2.reshape(B, C, S).astype(bf16)
    in_maps = [{"x1": np.ascontiguousarray(x1b[b]), "x2": x2p[b]} for b in range(B)]

    trace = bool(int(os.environ.get("CORR_TRACE", "0")))
    res = bass_utils.run_bass_kernel_spmd(
        nc, in_maps, core_ids=list(range(B)), trace=trace
    )
    if trace:
        _NC_CACHE["last_results"] = res
    out = np.stack([_extract(res.results[b]["gram"]) for b in range(B)], axis=0)
    return out.astype(np.float32)



# revision 2
# speedup vs baseline: 1.3892x; 1.3892x over previous
"""1D horizontal correlation (FlowNet cost volume, kernel_size=1) on 8 TRN2 cores.

out[b, d+4, y, x] = mean_c x1[b,c,y,x] * x2[b,c,y,x+d],  d in [-4, 4], OOB -> 0

Strategy (v2 — narrow-band via PE column tiling):
- Data-parallel over batch: B=8 -> one batch element per NeuronCore.
- Per core, flatten (H, W) -> S=30720 positions. C=128 = partition dim.
- Macro-tile = 128 positions, processed as FOUR col-tiled matmuls
  (tile_position=(0, 32j)): each loads 32 x1 positions as weights into
  array col-group j and streams a 40-col x2 window (32 + 2*MAXD halo):
      psum[32j + p', n] = sum_c x1[c, 128t+32j+p'] * x2[c, 128t+32j-4+n]
  The needed 9 displacements for row p' are psum[32j+p', p'..p'+8] — a
  40-wide band per 128 positions instead of the naive 136-wide gram
  (3.4x less PSUM->SBUF copy work, 3.4x less output DMA).
- 12 macro-tiles pack into one PSUM bank ([128, 480] fp32 = 1920B);
  ONE engine copy per bank extracts to SBUF bf16 (amortizes the
  ~120-170 cycle per-op engine overhead). Copies alternate DVE/ACT per
  4-bank chunk so each output DMA waits on a single engine.
- Host extracts the 9 diagonals from the [240, 128, 40] band and
  applies the zero mask for displacements crossing a row boundary.
- Inputs host-cast to bf16 with the 1/C scale folded into x1 (exact:
  C=128 is a power of two), halving input DMA traffic.

The TRN2 walrus build allows only ONE sync-wait per instruction:
- x1/x2 live in persistent SBUF tiles filled by disjoint slice DMAs on
  one HWDGE ring (no reuse -> DMAs carry no waits; FIFO order lets one
  wait subsume both x1 and x2 slice deps on a matmul).
- Before the first matmul into a RECYCLED psum bank, a 1-column junk
  LDWEIGHTS reads the sbuf region written by the copy that released
  that bank: the LDW absorbs the copy wait into PE's observed clock so
  the real matmul carries at most the one input-DMA wait.
- Output staging tiles are one-per-chunk (no reuse -> copies wait only
  on PE; out-DMAs wait only on the chunk's last copy).
"""

import os
import numpy as np

import concourse.bass as bass
import concourse.bacc as bacc
import concourse.mybir as mybir
import concourse.tile as tile
from concourse import bass_utils

B, C, H, W = 8, 128, 96, 320
S = H * W            # 30720 flattened positions per batch element
MAXD = 4
ND = 2 * MAXD + 1    # 9 displacement channels
TP = 128             # positions per macro-tile (PSUM partition dim)
NT = S // TP         # 240 macro-tiles
TPJ = 32             # positions per col-group sub-matmul
NG = TP // TPJ       # 4 col groups
WIN = TPJ + 2 * MAXD  # 40 band columns per col group
KPB = 12             # macro-tiles per PSUM bank (12*40*4B = 1920B < 2KB)
BPC = 4              # banks per output chunk
NCH = NT // (KPB * BPC)  # 5 output chunks
PSB = 8              # psum pool bufs (all 8 banks)
NSLICE = 16          # input DMA slices per tensor
SLICE = S // NSLICE  # 1920 positions

F32 = mybir.dt.float32
BF16 = mybir.dt.bfloat16


def _build_nc():
    nc = bacc.Bacc(debug=False)
    x1 = nc.dram_tensor("x1", [C, S], BF16, kind="ExternalInput")
    # x2 is host-padded with a zero halo of MAXD on both ends: [C, S + 8];
    # dram/sbuf col j = position j - MAXD.
    x2 = nc.dram_tensor("x2", [C, S + 2 * MAXD], BF16, kind="ExternalInput")
    gram = nc.dram_tensor("gram", [NCH, TP, BPC * KPB * WIN], BF16,
                          kind="ExternalOutput")

    with tile.TileContext(nc) as tc:
        with (
            tc.tile_pool(name="x1p", bufs=1) as x1p,
            tc.tile_pool(name="x2p", bufs=1) as x2p,
            tc.tile_pool(name="psp", bufs=PSB, space="PSUM") as psp,
            tc.tile_pool(name="outp", bufs=NCH) as outp,
        ):
            x1full = x1p.tile([C, S], BF16)
            x2full = x2p.tile([C, S + 2 * MAXD], BF16)
            for i in range(NSLICE):
                lo, hi = i * SLICE, (i + 1) * SLICE
                nc.scalar.dma_start(out=x1full[:, lo:hi], in_=x1[:, lo:hi])
                xhi = hi + 2 * MAXD if i == NSLICE - 1 else hi
                nc.scalar.dma_start(out=x2full[:, lo:xhi], in_=x2[:, lo:xhi])

            outts = []
            for ci in range(NCH):
                outt = outp.tile([TP, BPC * KPB * WIN], BF16)
                outts.append(outt)
                for b4 in range(BPC):
                    u = ci * BPC + b4          # global bank-use index
                    ps = psp.tile([TP, KPB * WIN], F32)
                    if u >= PSB:
                        # junk LDW: absorb the psum-release (copy) wait
                        pu = u - PSB
                        nc.tensor.ldweights(
                            weights=outts[pu // BPC][:, (pu % BPC) * KPB * WIN :
                                                     (pu % BPC) * KPB * WIN + 1]
                        )
                    for k in range(KPB):
                        t = u * KPB + k
                        for j in range(NG):
                            base = TP * t + TPJ * j
                            nc.tensor.matmul(
                                ps[TPJ * j : TPJ * (j + 1),
                                   WIN * k : WIN * (k + 1)],
                                lhsT=x1full[:, base : base + TPJ],
                                # x2full col = position + MAXD; window starts
                                # at position base - MAXD -> col base.
                                rhs=x2full[:, base : base + WIN],
                                start=True,
                                stop=True,
                                tile_position=(0, TPJ * j),
                            )
                    dst = outt[:, b4 * KPB * WIN : (b4 + 1) * KPB * WIN]
                    if ci % 2 == 0:
                        nc.vector.tensor_copy(dst, ps[:])
                    else:
                        nc.scalar.copy(dst, ps[:])

                nc.sync.dma_start(out=gram[ci], in_=outt[:])
    nc.compile()
    return nc


_NC_CACHE = {}


def _get_nc():
    if "nc" not in _NC_CACHE:
        _NC_CACHE["nc"] = _build_nc()
    return _NC_CACHE["nc"]


# host-side diagonal gather: band[t, p, j] = g[t, p, (p % 32) + j]
_P_IDX = np.arange(TP)
_COLS = (_P_IDX % TPJ)[:, None] + np.arange(ND)[None, :]  # [128, 9]


def _extract(gram: np.ndarray) -> np.ndarray:
    """gram [NCH, TP, BPC*KPB*WIN] -> out [ND, H, W] with OOB mask."""
    g = gram.reshape(NCH, TP, BPC, KPB, WIN)
    g = np.ascontiguousarray(g.transpose(0, 2, 3, 1, 4)).reshape(NT, TP, WIN)
    band9 = g[np.arange(NT)[:, None, None],
              _P_IDX[None, :, None],
              _COLS[None, :, :]]                    # [NT, TP, ND]
    out = np.asarray(band9, dtype=np.float32).reshape(S, ND)
    out = np.ascontiguousarray(out.T).reshape(ND, H, W)
    for j in range(ND):
        d = j - MAXD
        if d < 0:
            out[j, :, :-d] = 0.0
        elif d > 0:
            out[j, :, W - d:] = 0.0
    return out


def kernel(x1: np.ndarray, x2: np.ndarray) -> np.ndarray:
    assert x1.shape == (B, C, H, W) and x2.shape == (B, C, H, W)
    import ml_dtypes

    bf16 = ml_dtypes.bfloat16
    nc = _get_nc()
    # fold the 1/C mean scale into x1 (C = 128: exact exponent shift in bf16)
    x1b = (x1.reshape(B, C, S) * np.float32(1.0 / C)).astype(bf16)
    x2p = np.zeros((B, C, S + 2 * MAXD), dtype=bf16)
    x2p[:, :, MAXD : MAXD + S] = x2.reshape(B, C, S).astype(bf16)
    in_maps = [{"x1": np.ascontiguousarray(x1b[b]), "x2": x2p[b]} for b in range(B)]

    trace = bool(int(os.environ.get("CORR_TRACE", "0")))
    res = bass_utils.run_bass_kernel_spmd(
        nc, in_maps, core_ids=list(range(B)), trace=trace
    )
    if trace:
        _NC_CACHE["last_results"] = res
    out = np.stack([_extract(res.results[b]["gram"]) for b in range(B)], axis=0)
    return out.astype(np.float32)


# revision 3
# speedup vs baseline: 1.5451x; 1.1122x over previous
"""1D horizontal correlation (FlowNet cost volume, kernel_size=1) on 8 TRN2 cores.

out[b, d+4, y, x] = mean_c x1[b,c,y,x] * x2[b,c,y,x+d],  d in [-4, 4], OOB -> 0

Strategy (v3 — narrow-band via PE column tiling, ring-parallel DMA):
- Data-parallel over batch: B=8 -> one batch element per NeuronCore.
- Per core, flatten (H, W) -> S=30720 positions. C=128 = partition dim.
- Macro-tile = 128 positions, processed as FOUR col-tiled matmuls
  (tile_position=(0, 32j)): each loads 32 x1 positions as weights into
  array col-group j and streams a 40-col x2 window (32 + 2*MAXD halo):
      psum[32j + p', n] = sum_c x1[c, 128t+32j+p'] * x2[c, 128t+32j-4+n]
  The needed 9 displacements for row p' are psum[32j+p', p'..p'+8] — a
  40-wide band per 128 positions instead of the naive 136-wide gram.
  Measured: the 4 LDW+MM pairs of a macro-tile fully overlap (~40ns).
- 12 macro-tiles pack into one PSUM bank ([128, 480] fp32 = 1920B);
  ONE DVE copy per bank extracts to SBUF bf16.
- Engine/ring assignment (each HWDGE trigger costs ~720ns of descriptor
  generation, so the two rings split the input stream):
    ACT ring: x1 slice DMAs          SP ring: x2 slice DMAs
    DVE: all PSUM->SBUF copies       GPSIMD (SWDGE): output DMAs
- Slice sizes are graduated (small first) so PE starts early and the
  HBM pipe fills fast.
- Host extracts the 9 diagonals from the [240, 128, 40] band and
  applies the zero mask for displacements crossing a row boundary.
- Inputs host-cast to bf16 with the 1/C scale folded into x1 (exact:
  C=128 is a power of two), halving input DMA traffic.

The TRN2 walrus build allows only ONE sync-wait per instruction:
- x1/x2 SBUF tiles are persistent, filled by disjoint slice DMAs (no
  reuse -> DMA triggers carry no waits). Each x2 slice carries an 8-col
  halo overlap-free split so only slice-boundary-crossing matmuls need
  the (same-ring, FIFO-subsumed) later slice's sem.
- The LDWEIGHTS of each matmul carries the x1 wait; the MATMUL carries
  the x2 wait (weights dep subsumed by PE engine order after the LDW).
- Before the first matmul into a RECYCLED psum bank, a 1-column junk
  LDWEIGHTS reads the sbuf region written by the DVE copy that released
  that bank, absorbing the copy wait into PE's observed clock.
- Output staging tiles are one-per-chunk (no reuse -> copies wait only
  on PE; out-DMAs wait only on the chunk's last DVE copy).
"""

import os
import numpy as np

import concourse.bass as bass
import concourse.bacc as bacc
import concourse.mybir as mybir
import concourse.tile as tile
from concourse import bass_utils

B, C, H, W = 8, 128, 96, 320
S = H * W            # 30720 flattened positions per batch element
MAXD = 4
ND = 2 * MAXD + 1    # 9 displacement channels
TP = 128             # positions per macro-tile (PSUM partition dim)
NT = S // TP         # 240 macro-tiles
TPJ = 32             # positions per col-group sub-matmul
NG = TP // TPJ       # 4 col groups
WIN = TPJ + 2 * MAXD  # 40 band columns per col group
KPB = 12             # macro-tiles per PSUM bank (12*40*4B = 1920B < 2KB)
BPC = 2              # banks per output chunk
NCH = NT // (KPB * BPC)  # 10 output chunks
PSB = 8              # psum pool bufs (all 8 banks)
# input DMA slice sizes in positions (sum = S); small first so PE can
# start while the HBM pipe fills
SLICES = [256, 256, 512, 1024] + [2048] * 14
assert sum(SLICES) == S

F32 = mybir.dt.float32
BF16 = mybir.dt.bfloat16


def _build_nc():
    nc = bacc.Bacc(debug=False)
    x1 = nc.dram_tensor("x1", [C, S], BF16, kind="ExternalInput")
    # x2 is host-padded with a zero halo of MAXD on both ends: [C, S + 8];
    # dram/sbuf col j = position j - MAXD.
    x2 = nc.dram_tensor("x2", [C, S + 2 * MAXD], BF16, kind="ExternalInput")
    gram = nc.dram_tensor("gram", [NCH, TP, BPC * KPB * WIN], BF16,
                          kind="ExternalOutput")

    with tile.TileContext(nc) as tc:
        with (
            tc.tile_pool(name="x1p", bufs=1) as x1p,
            tc.tile_pool(name="x2p", bufs=1) as x2p,
            tc.tile_pool(name="psp", bufs=PSB, space="PSUM") as psp,
            tc.tile_pool(name="outp", bufs=NCH) as outp,
        ):
            x1full = x1p.tile([C, S], BF16)
            x2full = x2p.tile([C, S + 2 * MAXD], BF16)
            lo = 0
            for i, sz in enumerate(SLICES):
                hi = lo + sz
                nc.scalar.dma_start(out=x1full[:, lo:hi], in_=x1[:, lo:hi])
                xhi = hi + 2 * MAXD if i == len(SLICES) - 1 else hi
                nc.sync.dma_start(out=x2full[:, lo:xhi], in_=x2[:, lo:xhi])
                lo = hi

            outts = []
            for ci in range(NCH):
                outt = outp.tile([TP, BPC * KPB * WIN], BF16)
                outts.append(outt)
                for b2 in range(BPC):
                    u = ci * BPC + b2          # global bank-use index
                    ps = psp.tile([TP, KPB * WIN], F32)
                    if u >= PSB:
                        # junk LDW: absorb the psum-release (copy) wait
                        pu = u - PSB
                        nc.tensor.ldweights(
                            weights=outts[pu // BPC][:, (pu % BPC) * KPB * WIN :
                                                     (pu % BPC) * KPB * WIN + 1]
                        )
                    for k in range(KPB):
                        t = u * KPB + k
                        for j in range(NG):
                            base = TP * t + TPJ * j
                            nc.tensor.matmul(
                                ps[TPJ * j : TPJ * (j + 1),
                                   WIN * k : WIN * (k + 1)],
                                lhsT=x1full[:, base : base + TPJ],
                                # x2full col = position + MAXD; window starts
                                # at position base - MAXD -> col base.
                                rhs=x2full[:, base : base + WIN],
                                start=True,
                                stop=True,
                                tile_position=(0, TPJ * j),
                            )
                    nc.vector.tensor_copy(
                        outt[:, b2 * KPB * WIN : (b2 + 1) * KPB * WIN], ps[:]
                    )

                nc.gpsimd.dma_start(out=gram[ci], in_=outt[:])
    nc.compile()
    return nc


_NC_CACHE = {}


def _get_nc():
    if "nc" not in _NC_CACHE:
        _NC_CACHE["nc"] = _build_nc()
    return _NC_CACHE["nc"]


# host-side diagonal gather: band[t, p, j] = g[t, p, (p % 32) + j]
_P_IDX = np.arange(TP)
_COLS = (_P_IDX % TPJ)[:, None] + np.arange(ND)[None, :]  # [128, 9]


def _extract(gram: np.ndarray) -> np.ndarray:
    """gram [NCH, TP, BPC*KPB*WIN] -> out [ND, H, W] with OOB mask."""
    g = gram.reshape(NCH, TP, BPC, KPB, WIN)
    g = np.ascontiguousarray(g.transpose(0, 2, 3, 1, 4)).reshape(NT, TP, WIN)
    band9 = g[np.arange(NT)[:, None, None],
              _P_IDX[None, :, None],
              _COLS[None, :, :]]                    # [NT, TP, ND]
    out = np.asarray(band9, dtype=np.float32).reshape(S, ND)
    out = np.ascontiguousarray(out.T).reshape(ND, H, W)
    for j in range(ND):
        d = j - MAXD
        if d < 0:
            out[j, :, :-d] = 0.0
        elif d > 0:
            out[j, :, W - d:] = 0.0
    return out


def kernel(x1: np.ndarray, x2: np.ndarray) -> np.ndarray:
    assert x1.shape == (B, C, H, W) and x2.shape == (B, C, H, W)
    import ml_dtypes

    bf16 = ml_dtypes.bfloat16
    nc = _get_nc()
    # fold the 1/C mean scale into x1 (C = 128: exact exponent shift in bf16)
    x1b = (x1.reshape(B, C, S) * np.float32(1.0 / C)).astype(bf16)
    x2p = np.zeros((B, C, S + 2 * MAXD), dtype=bf16)
    x2p[:, :, MAXD : MAXD + S] = x2.reshape(B, C, S).astype(bf16)
    in_maps = [{"x1": np.ascontiguousarray(x1b[b]), "x2": x2p[b]} for b in range(B)]

    trace = bool(int(os.environ.get("CORR_TRACE", "0")))
    res = bass_utils.run_bass_kernel_spmd(
        nc, in_maps, core_ids=list(range(B)), trace=trace
    )
    if trace:
        _NC_CACHE["last_results"] = res
    out = np.stack([_extract(res.results[b]["gram"]) for b in range(B)], axis=0)
    return out.astype(np.float32)


# revision 7
# speedup vs baseline: 1.6312x; 1.0557x over previous
"""1D horizontal correlation (FlowNet cost volume, kernel_size=1) on 8 TRN2 cores.

out[b, d+4, y, x] = mean_c x1[b,c,y,x] * x2[b,c,y,x+d],  d in [-4, 4], OOB -> 0

Strategy (v3 — narrow-band via PE column tiling, ring-parallel DMA):
- Data-parallel over batch: B=8 -> one batch element per NeuronCore.
- Per core, flatten (H, W) -> S=30720 positions. C=128 = partition dim.
- Macro-tile = 128 positions, processed as FOUR col-tiled matmuls
  (tile_position=(0, 32j)): each loads 32 x1 positions as weights into
  array col-group j and streams a 40-col x2 window (32 + 2*MAXD halo):
      psum[32j + p', n] = sum_c x1[c, 128t+32j+p'] * x2[c, 128t+32j-4+n]
  The needed 9 displacements for row p' are psum[32j+p', p'..p'+8] — a
  40-wide band per 128 positions instead of the naive 136-wide gram.
  Measured: the 4 LDW+MM pairs of a macro-tile fully overlap (~40ns).
- 12 macro-tiles pack into one PSUM bank ([128, 480] fp32 = 1920B);
  ONE DVE copy per bank extracts to SBUF bf16.
- Engine/ring assignment (each HWDGE trigger costs ~720ns of descriptor
  generation, so the two rings split the input stream):
    ACT ring: x1 slice DMAs          SP ring: x2 slice DMAs + out DMAs
    DVE: all PSUM->SBUF copies       (GPSIMD unused: engaging it costs
                                      ~4us of extra preamble)
- Slice sizes are graduated (small first) so PE starts early and the
  HBM pipe fills fast.
- Host extracts the 9 diagonals from the [240, 128, 40] band and
  applies the zero mask for displacements crossing a row boundary.
- Inputs host-cast to bf16 with the 1/C scale folded into x1 (exact:
  C=128 is a power of two), halving input DMA traffic.

The TRN2 walrus build allows only ONE sync-wait per instruction:
- x1/x2 SBUF tiles are persistent, filled by disjoint slice DMAs (no
  reuse -> DMA triggers carry no waits). Each x2 slice carries an 8-col
  halo overlap-free split so only slice-boundary-crossing matmuls need
  the (same-ring, FIFO-subsumed) later slice's sem.
- The LDWEIGHTS of each matmul carries the x1 wait; the MATMUL carries
  the x2 wait (weights dep subsumed by PE engine order after the LDW).
- Before the first matmul into a RECYCLED psum bank, a 1-column junk
  LDWEIGHTS reads the sbuf region written by the DVE copy that released
  that bank, absorbing the copy wait into PE's observed clock.
- Output staging tiles are one-per-chunk (no reuse -> copies wait only
  on PE; out-DMAs wait only on the chunk's last DVE copy).
"""

import os
import numpy as np

import concourse.bass as bass
import concourse.bacc as bacc
import concourse.mybir as mybir
import concourse.tile as tile
from concourse import bass_utils

B, C, H, W = 8, 128, 96, 320
S = H * W            # 30720 flattened positions per batch element
MAXD = 4
ND = 2 * MAXD + 1    # 9 displacement channels
TP = 128             # positions per macro-tile (PSUM partition dim)
NT = S // TP         # 240 macro-tiles
TPJ = 32             # positions per col-group sub-matmul
NG = TP // TPJ       # 4 col groups
WIN = TPJ + 2 * MAXD  # 40 band columns per col group
KPB = 12             # macro-tiles per PSUM bank (12*40*4B = 1920B < 2KB)
BPC = 2              # banks per output chunk
NCH = NT // (KPB * BPC)  # 10 output chunks
PSB = 8              # psum pool bufs (all 8 banks)
# input DMA slice sizes in positions (sum = S); small first so PE can
# start while the HBM pipe fills, small last so the tail MMs wait on a
# small final transfer instead of a 1MB one
SLICES = [256, 256, 512, 1024] + [2048] * 13 + [1024, 512, 256, 256]
assert sum(SLICES) == S

F32 = mybir.dt.float32
BF16 = mybir.dt.bfloat16


def _build_nc():
    nc = bacc.Bacc(debug=False)
    x1 = nc.dram_tensor("x1", [C, S], BF16, kind="ExternalInput")
    # x2 is host-padded with a zero halo of MAXD on both ends: [C, S + 8];
    # dram/sbuf col j = position j - MAXD.
    x2 = nc.dram_tensor("x2", [C, S + 2 * MAXD], BF16, kind="ExternalInput")
    gram = nc.dram_tensor("gram", [NCH, TP, BPC * KPB * WIN], BF16,
                          kind="ExternalOutput")

    with tile.TileContext(nc) as tc:
        with (
            tc.tile_pool(name="x1p", bufs=1) as x1p,
            tc.tile_pool(name="x2p", bufs=1) as x2p,
            tc.tile_pool(name="psp", bufs=PSB, space="PSUM") as psp,
            tc.tile_pool(name="outp", bufs=NCH) as outp,
        ):
            x1full = x1p.tile([C, S], BF16)
            x2full = x2p.tile([C, S + 2 * MAXD], BF16)
            lo = 0
            for i, sz in enumerate(SLICES):
                hi = lo + sz
                # x2 first: it is the operand matmuls block on at slice
                # crossings (8-col halo read from the next slice)
                xhi = hi + 2 * MAXD if i == len(SLICES) - 1 else hi
                nc.sync.dma_start(out=x2full[:, lo:xhi], in_=x2[:, lo:xhi])
                nc.scalar.dma_start(out=x1full[:, lo:hi], in_=x1[:, lo:hi])
                lo = hi

            outts = []
            for ci in range(NCH):
                outt = outp.tile([TP, BPC * KPB * WIN], BF16)
                outts.append(outt)
                for b2 in range(BPC):
                    u = ci * BPC + b2          # global bank-use index
                    ps = psp.tile([TP, KPB * WIN], F32)
                    if u >= PSB:
                        # junk LDW: absorb the psum-release (copy) wait
                        pu = u - PSB
                        nc.tensor.ldweights(
                            weights=outts[pu // BPC][:, (pu % BPC) * KPB * WIN :
                                                     (pu % BPC) * KPB * WIN + 1]
                        )
                    for k in range(KPB):
                        t = u * KPB + k
                        for j in range(NG):
                            base = TP * t + TPJ * j
                            nc.tensor.matmul(
                                ps[TPJ * j : TPJ * (j + 1),
                                   WIN * k : WIN * (k + 1)],
                                lhsT=x1full[:, base : base + TPJ],
                                # x2full col = position + MAXD; window starts
                                # at position base - MAXD -> col base.
                                rhs=x2full[:, base : base + WIN],
                                start=True,
                                stop=True,
                                tile_position=(0, TPJ * j),
                            )
                    nc.vector.tensor_copy(
                        outt[:, b2 * KPB * WIN : (b2 + 1) * KPB * WIN], ps[:]
                    )

                if ci < NCH - 1:
                    nc.sync.dma_start(out=gram[ci], in_=outt[:])
                else:
                    # split the final chunk's DMA per bank so the last
                    # transfer (after the last copy) is half as large
                    half = KPB * WIN
                    nc.sync.dma_start(out=gram[ci, :, :half],
                                      in_=outt[:, :half])
                    nc.sync.dma_start(out=gram[ci, :, half:],
                                      in_=outt[:, half:])
    nc.compile()
    return nc


_NC_CACHE = {}


def _get_nc():
    if "nc" not in _NC_CACHE:
        _NC_CACHE["nc"] = _build_nc()
    return _NC_CACHE["nc"]


# host-side diagonal gather: band[t, p, j] = g[t, p, (p % 32) + j]
_P_IDX = np.arange(TP)
_COLS = (_P_IDX % TPJ)[:, None] + np.arange(ND)[None, :]  # [128, 9]


def _extract(gram: np.ndarray) -> np.ndarray:
    """gram [NCH, TP, BPC*KPB*WIN] -> out [ND, H, W] with OOB mask."""
    g = gram.reshape(NCH, TP, BPC, KPB, WIN)
    g = np.ascontiguousarray(g.transpose(0, 2, 3, 1, 4)).reshape(NT, TP, WIN)
    band9 = g[np.arange(NT)[:, None, None],
              _P_IDX[None, :, None],
              _COLS[None, :, :]]                    # [NT, TP, ND]
    out = np.asarray(band9, dtype=np.float32).reshape(S, ND)
    out = np.ascontiguousarray(out.T).reshape(ND, H, W)
    for j in range(ND):
        d = j - MAXD
        if d < 0:
            out[j, :, :-d] = 0.0
        elif d > 0:
            out[j, :, W - d:] = 0.0
    return out


def kernel(x1: np.ndarray, x2: np.ndarray) -> np.ndarray:
    assert x1.shape == (B, C, H, W) and x2.shape == (B, C, H, W)
    import ml_dtypes

    bf16 = ml_dtypes.bfloat16
    nc = _get_nc()
    # fold the 1/C mean scale into x1 (C = 128: exact exponent shift in bf16)
    x1b = (x1.reshape(B, C, S) * np.float32(1.0 / C)).astype(bf16)
    x2p = np.zeros((B, C, S + 2 * MAXD), dtype=bf16)
    x2p[:, :, MAXD : MAXD + S] = x2.reshape(B, C, S).astype(bf16)
    in_maps = [{"x1": np.ascontiguousarray(x1b[b]), "x2": x2p[b]} for b in range(B)]

    trace = bool(int(os.environ.get("CORR_TRACE", "0")))
    res = bass_utils.run_bass_kernel_spmd(
        nc, in_maps, core_ids=list(range(B)), trace=trace
    )
    if trace:
        _NC_CACHE["last_results"] = res
    out = np.stack([_extract(res.results[b]["gram"]) for b in range(B)], axis=0)
    return out.astype(np.float32)


# revision 8
# speedup vs baseline: 1.6421x; 1.0067x over previous
"""1D horizontal correlation (FlowNet cost volume, kernel_size=1) on 8 TRN2 cores.

out[b, d+4, y, x] = mean_c x1[b,c,y,x] * x2[b,c,y,x+d],  d in [-4, 4], OOB -> 0

Strategy (v3 — narrow-band via PE column tiling, ring-parallel DMA):
- Data-parallel over batch: B=8 -> one batch element per NeuronCore.
- Per core, flatten (H, W) -> S=30720 positions. C=128 = partition dim.
- Macro-tile = 128 positions, processed as FOUR col-tiled matmuls
  (tile_position=(0, 32j)): each loads 32 x1 positions as weights into
  array col-group j and streams a 40-col x2 window (32 + 2*MAXD halo):
      psum[32j + p', n] = sum_c x1[c, 128t+32j+p'] * x2[c, 128t+32j-4+n]
  The needed 9 displacements for row p' are psum[32j+p', p'..p'+8] — a
  40-wide band per 128 positions instead of the naive 136-wide gram.
  Measured: the 4 LDW+MM pairs of a macro-tile fully overlap (~40ns).
- 12 macro-tiles pack into one PSUM bank ([128, 480] fp32 = 1920B);
  ONE DVE copy per bank extracts to SBUF bf16.
- Engine/ring assignment (each HWDGE trigger costs ~720ns of descriptor
  generation, so the two rings split the input stream):
    ACT ring: x1 slice DMAs          SP ring: x2 slice DMAs + out DMAs
    DVE: all PSUM->SBUF copies       (GPSIMD unused: engaging it costs
                                      ~4us of extra preamble)
- Slice sizes are graduated (small first) so PE starts early and the
  HBM pipe fills fast.
- Host extracts the 9 diagonals from the [240, 128, 40] band and
  applies the zero mask for displacements crossing a row boundary.
- Inputs host-cast to bf16 with the 1/C scale folded into x1 (exact:
  C=128 is a power of two), halving input DMA traffic.

The TRN2 walrus build allows only ONE sync-wait per instruction:
- x1/x2 SBUF tiles are persistent, filled by disjoint slice DMAs (no
  reuse -> DMA triggers carry no waits). Each x2 slice carries an 8-col
  halo overlap-free split so only slice-boundary-crossing matmuls need
  the (same-ring, FIFO-subsumed) later slice's sem.
- The LDWEIGHTS of each matmul carries the x1 wait; the MATMUL carries
  the x2 wait (weights dep subsumed by PE engine order after the LDW).
- Before the first matmul into a RECYCLED psum bank, a 1-column junk
  LDWEIGHTS reads the sbuf region written by the DVE copy that released
  that bank, absorbing the copy wait into PE's observed clock.
- Output staging tiles are one-per-chunk (no reuse -> copies wait only
  on PE; out-DMAs wait only on the chunk's last DVE copy).
"""

import os
import numpy as np

import concourse.bass as bass
import concourse.bacc as bacc
import concourse.mybir as mybir
import concourse.tile as tile
from concourse import bass_utils

B, C, H, W = 8, 128, 96, 320
S = H * W            # 30720 flattened positions per batch element
MAXD = 4
ND = 2 * MAXD + 1    # 9 displacement channels
TP = 128             # positions per macro-tile (PSUM partition dim)
NT = S // TP         # 240 macro-tiles
TPJ = 32             # positions per col-group sub-matmul
NG = TP // TPJ       # 4 col groups
WIN = TPJ + 2 * MAXD  # 40 band columns per col group
KPB = 12             # macro-tiles per PSUM bank (12*40*4B = 1920B < 2KB)
BPC = 2              # banks per output chunk
NCH = NT // (KPB * BPC)  # 10 output chunks
PSB = 8              # psum pool bufs (all 8 banks)
# input DMA slice sizes in positions (sum = S); small first so PE can
# start while the HBM pipe fills, small last so the tail MMs wait on a
# small final transfer instead of a 1MB one
SLICES = [256, 256, 512, 1024] + [2048] * 13 + [1024, 512, 256, 256]
assert sum(SLICES) == S

F32 = mybir.dt.float32
BF16 = mybir.dt.bfloat16


def _build_nc():
    nc = bacc.Bacc(debug=False)
    x1 = nc.dram_tensor("x1", [C, S], BF16, kind="ExternalInput")
    # x2 is host-padded with a zero halo of MAXD on both ends: [C, S + 8];
    # dram/sbuf col j = position j - MAXD.
    x2 = nc.dram_tensor("x2", [C, S + 2 * MAXD], BF16, kind="ExternalInput")
    gram = nc.dram_tensor("gram", [NCH, TP, BPC * KPB * WIN], BF16,
                          kind="ExternalOutput")

    with tile.TileContext(nc) as tc:
        with (
            tc.tile_pool(name="x1p", bufs=1) as x1p,
            tc.tile_pool(name="x2p", bufs=1) as x2p,
            tc.tile_pool(name="psp", bufs=PSB, space="PSUM") as psp,
            tc.tile_pool(name="outp", bufs=NCH) as outp,
        ):
            x1full = x1p.tile([C, S], BF16)
            x2full = x2p.tile([C, S + 2 * MAXD], BF16)
            lo = 0
            for i, sz in enumerate(SLICES):
                hi = lo + sz
                # x2 first: it is the operand matmuls block on at slice
                # crossings (8-col halo read from the next slice)
                xhi = hi + 2 * MAXD if i == len(SLICES) - 1 else hi
                nc.sync.dma_start(out=x2full[:, lo:xhi], in_=x2[:, lo:xhi])
                nc.scalar.dma_start(out=x1full[:, lo:hi], in_=x1[:, lo:hi])
                lo = hi

            outts = []
            for ci in range(NCH):
                outt = outp.tile([TP, BPC * KPB * WIN], BF16)
                outts.append(outt)
                for b2 in range(BPC):
                    u = ci * BPC + b2          # global bank-use index
                    ps = psp.tile([TP, KPB * WIN], F32)
                    if u >= PSB:
                        # junk LDW: absorb the psum-release (copy) wait
                        pu = u - PSB
                        nc.tensor.ldweights(
                            weights=outts[pu // BPC][:, (pu % BPC) * KPB * WIN :
                                                     (pu % BPC) * KPB * WIN + 1]
                        )
                    for k in range(KPB):
                        t = u * KPB + k
                        for j in range(NG):
                            base = TP * t + TPJ * j
                            nc.tensor.matmul(
                                ps[TPJ * j : TPJ * (j + 1),
                                   WIN * k : WIN * (k + 1)],
                                lhsT=x1full[:, base : base + TPJ],
                                # x2full col = position + MAXD; window starts
                                # at position base - MAXD -> col base.
                                rhs=x2full[:, base : base + WIN],
                                start=True,
                                stop=True,
                                tile_position=(0, TPJ * j),
                            )
                    nc.vector.tensor_copy(
                        outt[:, b2 * KPB * WIN : (b2 + 1) * KPB * WIN], ps[:]
                    )

                # SWDGE (gpsimd) keeps output triggers off the two HWDGE
                # rings: an out trigger waiting on its copy would block
                # later input-slice triggers queued behind it (FIFO).
                if ci < NCH - 1:
                    nc.gpsimd.dma_start(out=gram[ci], in_=outt[:])
                else:
                    # split the final chunk's DMA per bank so the last
                    # transfer (after the last copy) is half as large
                    half = KPB * WIN
                    nc.gpsimd.dma_start(out=gram[ci, :, :half],
                                        in_=outt[:, :half])
                    nc.gpsimd.dma_start(out=gram[ci, :, half:],
                                        in_=outt[:, half:])
    nc.compile()
    return nc


_NC_CACHE = {}


def _get_nc():
    if "nc" not in _NC_CACHE:
        _NC_CACHE["nc"] = _build_nc()
    return _NC_CACHE["nc"]


# host-side diagonal gather: band[t, p, j] = g[t, p, (p % 32) + j]
_P_IDX = np.arange(TP)
_COLS = (_P_IDX % TPJ)[:, None] + np.arange(ND)[None, :]  # [128, 9]


def _extract(gram: np.ndarray) -> np.ndarray:
    """gram [NCH, TP, BPC*KPB*WIN] -> out [ND, H, W] with OOB mask."""
    g = gram.reshape(NCH, TP, BPC, KPB, WIN)
    g = np.ascontiguousarray(g.transpose(0, 2, 3, 1, 4)).reshape(NT, TP, WIN)
    band9 = g[np.arange(NT)[:, None, None],
              _P_IDX[None, :, None],
              _COLS[None, :, :]]                    # [NT, TP, ND]
    out = np.asarray(band9, dtype=np.float32).reshape(S, ND)
    out = np.ascontiguousarray(out.T).reshape(ND, H, W)
    for j in range(ND):
        d = j - MAXD
        if d < 0:
            out[j, :, :-d] = 0.0
        elif d > 0:
            out[j, :, W - d:] = 0.0
    return out


def kernel(x1: np.ndarray, x2: np.ndarray) -> np.ndarray:
    assert x1.shape == (B, C, H, W) and x2.shape == (B, C, H, W)
    import ml_dtypes

    bf16 = ml_dtypes.bfloat16
    nc = _get_nc()
    # fold the 1/C mean scale into x1 (C = 128: exact exponent shift in bf16)
    x1b = (x1.reshape(B, C, S) * np.float32(1.0 / C)).astype(bf16)
    x2p = np.zeros((B, C, S + 2 * MAXD), dtype=bf16)
    x2p[:, :, MAXD : MAXD + S] = x2.reshape(B, C, S).astype(bf16)
    in_maps = [{"x1": np.ascontiguousarray(x1b[b]), "x2": x2p[b]} for b in range(B)]

    trace = bool(int(os.environ.get("CORR_TRACE", "0")))
    res = bass_utils.run_bass_kernel_spmd(
        nc, in_maps, core_ids=list(range(B)), trace=trace
    )
    if trace:
        _NC_CACHE["last_results"] = res
    out = np.stack([_extract(res.results[b]["gram"]) for b in range(B)], axis=0)
    return out.astype(np.float32)
